# revision 30
# baseline (speedup 1.0000x reference)
"""GRU decoder Trainium2 kernel (data-parallel over batch, 8 cores).

Reference (per step t, PyTorch nn.GRU gate order r,z,n):
    gi = x @ w_ih.T + b_ih ; gh = h @ w_hh.T + b_hh
    r = sig(i_r + h_r); z = sig(i_z + h_z); n = tanh(i_n + r * h_n)
    h' = (1-z)*n + z*h ; y = h' @ w_fc.T + b_fc ; x <- y
Shapes: H=1024, O=768, B=256, T=256.  Each core handles 32 batch rows.

Structure (v6 - transposed state):
  * x_t = y_{t-1} folds into the hidden-side matmuls, so every recurrent
    matmul contracts over H=1024: regions r, hn (= h_n), z, in (= i_n).
  * The state lives ONLY as hsb = h'^T (bf16, PE lhsT layout).  The
    chain computes zs/n in normal layout, transposes zs and n (PE,
    cheap, off the critical tail), then finishes in transposed space:
        hsb' = n^T (1 - z^T) + z^T hsb
    so NOTHING follows the last vector op before the next gate matmuls.
  * Gate PSUM: pair tile [r|hn] computed with single N=512 matmuls
    (fewer LDWEIGHTS/issues) + separate z / in banks (separate banks =
    per-region dependency granularity, so zs runs before the in-region
    finishes), all double-buffered.
  * Biases seed PSUM via K=1 ones-row matmuls issued in the PE-idle
    chain window.  One start=True per bank strip only - a second start
    in the same strip clears has_written and loses the earlier bias.
  * y_t matmuls + zs/n transposes share PSUM banks with the chain
    scratch; y and bias MMs hide in the chain window.
  * Step-0 gates come from the host; b_fc is added on the host.
"""

import numpy as np
import ml_dtypes

import concourse.bass as bass
import concourse.bacc as bacc
import concourse.tile as tile
from concourse import mybir
from concourse.bass_utils import run_bass_kernel_spmd

H = 1024
O = 768
B = 256
T = 256
NCORES = 8
BC = B // NCORES  # 32 batch rows per core

KH = H // 128  # 8 contraction chunks
NGATE = 4      # regions r, hn, z, in (issue order)
YW = O // 4    # 192 y cols per quadrant

F32 = mybir.dt.float32
BF16 = mybir.dt.bfloat16
AF = mybir.ActivationFunctionType
ALU = mybir.AluOpType

_COMPILED = None

# bf16 const layout: WG | WF | ONES | BIAS
WG_N = NGATE * KH * 4 * 256   # 32768
WF_N = KH * 4 * YW            # 6144
NB = WG_N + WF_N + 32 + 4096  # 43040
# f32 const layout: G0 (r|hn|z|in) | H0T | IDT
NF = NGATE * 256 + 256 + 128  # 1408


def _hslice(hsb, k):
    """lhsT chunk k (h features 128k..128k+128) from packed h'^T tile."""
    c = 128 * (k % 2) + 32 * (k // 2)
    return hsb[:, c : c + 32]


def _build_nc():
    nc = bacc.Bacc("TRN2", target_bir_lowering=False, debug=False, num_devices=NCORES)

    cb = nc.declare_dram_parameter("CB", [128, NB], BF16, isOutput=False)
    cf = nc.declare_dram_parameter("CF", [128, NF], F32, isOutput=False)
    o = nc.declare_dram_parameter("O", [T, 128, YW], F32, isOutput=True)

    with tile.TileContext(nc) as tc:
        with (
            tc.tile_pool(name="wpool", bufs=1) as wpool,
            tc.tile_pool(name="state", bufs=2) as spool,
            tc.tile_pool(name="act", bufs=2) as apool,
            tc.tile_pool(name="gps", bufs=2, space="PSUM") as gpool,
            tc.tile_pool(name="tps", bufs=1, space="PSUM") as tpool,
        ):
            CB = wpool.tile([128, NB], BF16, tag="CB")
            CF = wpool.tile([128, NF], F32, tag="CF")
            nc.sync.dma_start(CB[:], cb[:])
            nc.sync.dma_start(CF[:], cf[:])
            WG = CB[:, 0:WG_N]
            WF = CB[:, WG_N : WG_N + WF_N]
            ONES = CB[0:1, WG_N + WF_N : WG_N + WF_N + 32]
            BIAS = CB[0:1, WG_N + WF_N + 32 : NB]
            G0 = CF[:, 0 : NGATE * 256]
            H0T = CF[:, NGATE * 256 : NGATE * 256 + 256]
            IDT = CF[:, NGATE * 256 + 256 : NF]

            def chain_partA(r_src, hn_src, z_src, in_src):
                """scalar: rs, zs, tanh; vector: rt, ns; PE: zs^T.
                Also allocates tpN ([0:256] for n^T, [256:448] for y)."""
                rs = apool.tile([128, 256], F32, tag="rs")
                nc.scalar.activation(rs[:], r_src, AF.Sigmoid)
                zs = apool.tile([128, 256], F32, tag="zs")
                nc.scalar.activation(zs[:], z_src, AF.Sigmoid)
                rt = apool.tile([128, 256], F32, tag="rt")
                nc.vector.tensor_tensor(rt[:], rs[:], hn_src, ALU.mult)
                ns = apool.tile([128, 256], F32, tag="ns")
                nc.vector.tensor_tensor(ns[:], rt[:], in_src, ALU.add)
                tpZ = tpool.tile([128, 256], F32, tag="tpZ")
                nc.tensor.transpose(tpZ[:, 0:128], zs[:, 0:128], IDT)
                nc.tensor.transpose(tpZ[:, 128:256], zs[:, 128:256], IDT)
                zcT = apool.tile([128, 256], F32, tag="zcT")
                nc.scalar.activation(zcT[:], tpZ[:], AF.Copy, bias=1.0, scale=-1.0)
                n = apool.tile([128, 256], F32, tag="n")
                nc.scalar.activation(n[:], ns[:], AF.Tanh)
                tpN = tpool.tile([128, 448], F32, tag="tpN")
                return n, tpZ, zcT, tpN

            def chain_partB(n, tpZ, zcT, tpN, hsb_prev):
                """PE: n^T; vector: p^T, v^T, hsb' (bf16)."""
                nc.tensor.transpose(tpN[:, 0:128], n[:, 0:128], IDT)
                nc.tensor.transpose(tpN[:, 128:256], n[:, 128:256], IDT)
                pT = apool.tile([128, 256], F32, tag="pT")
                nc.vector.tensor_tensor(pT[:], tpZ[:], hsb_prev, ALU.mult)
                vT = apool.tile([128, 256], F32, tag="vT")
                nc.vector.tensor_tensor(vT[:], tpN[:, 0:256], zcT[:], ALU.mult)
                hsb2 = spool.tile([128, 256], BF16, tag="hsb")
                nc.vector.tensor_tensor(hsb2[:], vT[:], pT[:], ALU.add)
                return hsb2

            def emit_y(hsb_t, tpN):
                for k in range(KH):
                    lhsT = _hslice(hsb_t, k)
                    for j in range(4):
                        wofs = (k * 4 + j) * YW
                        nc.tensor.matmul(
                            tpN[32 * j : 32 * j + 32, 256:448],
                            lhsT,
                            WF[:, wofs : wofs + YW],
                            start=(k == 0),
                            stop=(k == KH - 1),
                            tile_position=(0, 32 * j),
                        )

            # step 0: gates computed host-side (biases already included)
            n0, tpZ0, zcT0, tpN0 = chain_partA(
                G0[:, 0:256], G0[:, 256:512], G0[:, 512:768], G0[:, 768:1024]
            )
            hsb = chain_partB(n0, tpZ0, zcT0, tpN0, H0T)

            for t in range(T):
                last = t == T - 1
                if not last:
                    # gates for step t+1, read hsb_t
                    gA = gpool.tile([128, 512], F32, tag="gA")  # r | hn
                    gZ = gpool.tile([128, 256], F32, tag="gZ")
                    gI = gpool.tile([128, 256], F32, tag="gI")
                    # bias seeds (start=True).  gA: ONE N=512 MM per
                    # quadrant covering r|hn together (a second start in
                    # the same bank strip would clear the earlier bias).
                    for j in range(4):
                        nc.tensor.matmul(
                            gA[32 * j : 32 * j + 32, :],
                            ONES[:, 0:32],
                            BIAS[:, 512 * j : 512 * j + 512],
                            start=True, stop=False, tile_position=(0, 32 * j),
                        )
                    for gi, gt in ((2, gZ), (3, gI)):
                        for j in range(4):
                            bofs = 1024 * gi + 256 * j
                            nc.tensor.matmul(
                                gt[32 * j : 32 * j + 32, :],
                                ONES[:, 0:32],
                                BIAS[:, bofs : bofs + 256],
                                start=True, stop=False, tile_position=(0, 32 * j),
                            )
                    # r|hn pair as single N=512 matmuls (fewer LDW/issues)
                    for k in range(KH):
                        lhsT = _hslice(hsb, k)
                        for j in range(4):
                            wofs = (k * 4 + j) * 512
                            nc.tensor.matmul(
                                gA[32 * j : 32 * j + 32, :],
                                lhsT,
                                WG[:, wofs : wofs + 512],
                                start=False,
                                stop=(k == KH - 1),
                                tile_position=(0, 32 * j),
                            )
                    # then z, in regions (N=256, own banks)
                    for gi, gt in ((0, gZ), (1, gI)):
                        for k in range(KH):
                            lhsT = _hslice(hsb, k)
                            for j in range(4):
                                wofs = 16384 + ((gi * KH + k) * 4 + j) * 256
                                nc.tensor.matmul(
                                    gt[32 * j : 32 * j + 32, :],
                                    lhsT,
                                    WG[:, wofs : wofs + 256],
                                    start=False,
                                    stop=(k == KH - 1),
                                    tile_position=(0, 32 * j),
                                )
                    # chain for step t+1 (reads this cycle's PSUM); y_t
                    # goes between zs^T and n^T on the PE queue.
                    nA, tpZA, zcTA, tpNA = chain_partA(
                        gA[:, 0:256], gA[:, 256:512], gZ[:], gI[:]
                    )
                    emit_y(hsb, tpNA)
                    hsb = chain_partB(nA, tpZA, zcTA, tpNA, hsb[:])
                    ysrc = tpNA
                else:
                    tpN_last = tpool.tile([128, 448], F32, tag="tpN")
                    emit_y(hsb, tpN_last)
                    ysrc = tpN_last

                ys = apool.tile([128, YW], F32, tag="ys")
                nc.vector.tensor_copy(ys[:], ysrc[:, 256:448])
                nc.sync.dma_start(o[t], ys[:])

    nc.compile()
    return nc


def _pack_bat(M):
    """[32, 4*W] -> [128, W]: row 32j+b holds M[b, W*j : W*j+W]."""
    w = M.shape[1] // 4
    return np.ascontiguousarray(
        M.reshape(BC, 4, w).transpose(1, 0, 2).reshape(128, w)
    )


def _prep_shared(w_ih, w_hh, b_ih, b_hh, w_fc, b_fc):
    wihT = w_ih.T.astype(np.float64)  # [768, 3072]
    whhT = w_hh.T.astype(np.float64)  # [1024, 3072]
    wfcT = w_fc.T.astype(np.float64)  # [1024, 768]
    fold = wfcT @ wihT                # [1024, 3072]
    Wr = fold[:, 0:H] + whhT[:, 0:H]
    Wz = fold[:, H : 2 * H] + whhT[:, H : 2 * H]
    Win = fold[:, 2 * H : 3 * H]
    Whn = whhT[:, 2 * H : 3 * H]

    bfold = b_fc.astype(np.float64) @ wihT  # [3072]
    br = bfold[0:H] + b_ih[0:H] + b_hh[0:H]
    bz = bfold[H : 2 * H] + b_ih[H : 2 * H] + b_hh[H : 2 * H]
    bin_ = bfold[2 * H :] + b_ih[2 * H :]
    bhn = b_hh[2 * H :].astype(np.float64)

    blocks = []
    # r|hn interleaved per (k,j) for N=512 pair matmuls
    for k in range(KH):
        for j in range(4):
            blocks.append(Wr[128 * k : 128 * k + 128, 256 * j : 256 * j + 256])
            blocks.append(Whn[128 * k : 128 * k + 128, 256 * j : 256 * j + 256])
    # then z, in blocks (N=256)
    for G in (Wz, Win):
        for k in range(KH):
            for j in range(4):
                blocks.append(G[128 * k : 128 * k + 128, 256 * j : 256 * j + 256])
    WGp = np.concatenate(blocks, axis=1).astype(ml_dtypes.bfloat16)  # [128, 32768]

    yblocks = []
    for k in range(KH):
        for j in range(4):
            yblocks.append(wfcT[128 * k : 128 * k + 128, YW * j : YW * j + YW])
    WFp = np.concatenate(yblocks, axis=1).astype(ml_dtypes.bfloat16)  # [128, 6144]

    ones_col = np.zeros((128, 32), ml_dtypes.bfloat16)
    ones_col[0, :] = 1
    # bias layout: j-paired [br_j | bhn_j] (4x512) then bz (1024), bin (1024)
    bias_row = np.empty(4096, np.float64)
    for j in range(4):
        bias_row[512 * j : 512 * j + 256] = br[256 * j : 256 * j + 256]
        bias_row[512 * j + 256 : 512 * j + 512] = bhn[256 * j : 256 * j + 256]
    bias_row[2048:3072] = bz
    bias_row[3072:4096] = bin_
    bias_col = np.zeros((128, 4096), ml_dtypes.bfloat16)
    bias_col[0, :] = bias_row.astype(ml_dtypes.bfloat16)

    CBp = np.concatenate([WGp, WFp, ones_col, bias_col], axis=1)  # [128, NB]
    assert CBp.shape[1] == NB
    IDT = np.eye(128, dtype=np.float32)
    return CBp, IDT


def _build_in_maps(inputs):
    src = np.asarray(inputs["src"], np.float32)
    hidden = np.asarray(inputs["hidden"], np.float32)
    w_ih = np.asarray(inputs["w_ih"], np.float32)
    w_hh = np.asarray(inputs["w_hh"], np.float32)
    b_ih = np.asarray(inputs["b_ih"], np.float32)
    b_hh = np.asarray(inputs["b_hh"], np.float32)
    w_fc = np.asarray(inputs["w_fc"], np.float32)
    b_fc = np.asarray(inputs["b_fc"], np.float32)

    CBp, IDT = _prep_shared(w_ih, w_hh, b_ih, b_hh, w_fc, b_fc)

    # step-0 gates on host (f64): from x0=src[0], h0=hidden[0]
    x0 = src[0].astype(np.float64)
    h0 = hidden[0].astype(np.float64)
    gi0 = x0 @ w_ih.T.astype(np.float64) + b_ih.astype(np.float64)
    gh0 = h0 @ w_hh.T.astype(np.float64) + b_hh.astype(np.float64)
    g0r = gi0[:, 0:H] + gh0[:, 0:H]
    g0z = gi0[:, H : 2 * H] + gh0[:, H : 2 * H]
    g0in = gi0[:, 2 * H :]
    g0hn = gh0[:, 2 * H :]

    in_maps = []
    for c in range(NCORES):
        sl = slice(BC * c, BC * (c + 1))
        G0 = np.concatenate(
            [
                _pack_bat(g0r[sl]),
                _pack_bat(g0hn[sl]),
                _pack_bat(g0z[sl]),
                _pack_bat(g0in[sl]),
            ],
            axis=1,
        )  # [128, 1024] in region order r|hn|z|in
        HP0 = _pack_bat(h0[sl])  # [128, 256]
        H0T = np.concatenate(
            [HP0[:, 0:128].T, HP0[:, 128:256].T], axis=1
        )  # transposed-state layout
        CFp = np.concatenate([G0, H0T, IDT], axis=1).astype(np.float32)
        assert CFp.shape[1] == NF
        in_maps.append(dict(CB=CBp, CF=CFp))
    return in_maps


def kernel(src, tgt, hidden, w_ih, w_hh, b_ih, b_hh, w_fc, b_fc, **_kw):
    global _COMPILED
    b_fc = np.asarray(b_fc, np.float32)

    if _COMPILED is None:
        _COMPILED = _build_nc()
    nc = _COMPILED

    in_maps = _build_in_maps(
        dict(src=src, hidden=hidden, w_ih=w_ih, w_hh=w_hh, b_ih=b_ih,
             b_hh=b_hh, w_fc=w_fc, b_fc=b_fc)
    )

    res = run_bass_kernel_spmd(nc, in_maps, list(range(NCORES)))

    out = np.empty((T, B, O), np.float32)
    for c in range(NCORES):
        sl = slice(BC * c, BC * (c + 1))
        oc = np.asarray(res.results[c]["O"])  # [T, 128, 192]
        out[:, sl, :] = (
            oc.reshape(T, 4, BC, YW).transpose(0, 2, 1, 3).reshape(T, BC, O)
        )
    out += b_fc[None, None, :]
    return out
